# revision 18
# baseline (speedup 1.0000x reference)
"""MoE routing kernel for Trainium2, 8 NeuronCores, expert-parallel.

Reference: E=8 experts (top-2 gating), each expert = per-variable 2-head
self-attention over time + 2-layer MLP; combine = log(sum_e g_e*exp(out_e)).

Strategy (dense expert-parallel, one expert per core):
- Host preps per-core inputs: x transposed to [N, D, B*T] bf16, this core's
  expert weights (bf16 attention weights / f32 MLP weights), fused biases.
- Algebraic simplifications baked in:
  * key bias bd[e,0] shifts every score of a query equally -> softmax
    invariant -> dropped.
  * value bias bd[e,1] passes through softmax (rows sum to 1) -> folded into
    the first MLP bias on host: bs0' = bs0 + bd1[n] @ Ws0.
  * RS payload is g*(exp(o)-1) in bf16 (values ~0.01 -> tiny rounding), then
    out = log1p(sum) after the ReduceScatter; gates sum to 1 exactly.
- Device: gating (mean->logits->top2 via max/2nd-max masking) computed from a
  dedicated re-read of x so it is ready early; per-variable pipeline of
  k/v projections (bf16), 2-head attention with b-parity/head PSUM quadrant
  packing, softmax denominator via PE column-sum matmul + DRAM-bounce
  reciprocal broadcast, MLP in float32r, exp epilogue; 4 chunked bf16
  ReduceScatters overlap the tail; log1p on-device.
- Output is n-sharded across cores; host reassembles + transposes back.
"""

import numpy as np
import ml_dtypes

from concourse import bass, bacc, tile, mybir
from concourse.bass_utils import run_bass_kernel_spmd

E, K = 8, 2
B, T, N, D = 32, 64, 32, 128
H = 2
HD = D // H          # 64
BT = B * T           # 2048
N_CORES = 8
CORE_IDS = list(range(N_CORES))

F32 = mybir.dt.float32
F32R = mybir.dt.float32r
BF16 = mybir.dt.bfloat16
AF = mybir.ActivationFunctionType
ALU = mybir.AluOpType

_cache = {}


def build():
    nc = bacc.Bacc("TRN2", target_bir_lowering=False, debug=False,
                   num_devices=N_CORES)

    # ---- external inputs (per core) ----
    xT_e = nc.dram_tensor("xT", [N, D, BT], BF16, kind="ExternalInput")
    wd0_e = nc.dram_tensor("wd0", [N, D, D], BF16, kind="ExternalInput")
    wd1_e = nc.dram_tensor("wd1", [N, D, D], BF16, kind="ExternalInput")
    ws0_e = nc.dram_tensor("ws0", [D, D], BF16, kind="ExternalInput")
    ws1_e = nc.dram_tensor("ws1", [D, D], BF16, kind="ExternalInput")
    b0_e = nc.dram_tensor("b0", [D, N], F32, kind="ExternalInput")
    b1_e = nc.dram_tensor("b1", [D, 1], F32, kind="ExternalInput")
    wg_e = nc.dram_tensor("wg", [D, E], F32, kind="ExternalInput")
    oh_e = nc.dram_tensor("oh", [B, E], F32, kind="ExternalInput")
    id32_e = nc.dram_tensor("id32", [B, B], F32, kind="ExternalInput")
    pmask_e = nc.dram_tensor("pmask", [D, 32], BF16, kind="ExternalInput")
    out_e = nc.dram_tensor("out", [N // N_CORES, D, BT], F32,
                           kind="ExternalOutput")

    with tile.TileContext(nc) as tc:
        with (
            nc.allow_low_precision(reason="bf16 intermediates by design"),
            tc.tile_pool(name="const", bufs=1) as cpool,
            tc.tile_pool(name="xa", bufs=6) as xapool,
            tc.tile_pool(name="xn", bufs=4) as xnpool,
            tc.tile_pool(name="kv", bufs=3) as kvpool,
            tc.tile_pool(name="att", bufs=3) as attpool,
            tc.tile_pool(name="mlp", bufs=2) as mlppool,
            tc.tile_pool(name="ep", bufs=2) as eppool,
            tc.tile_pool(name="dring", bufs=6) as dpool,
            tc.tile_pool(name="sm", bufs=1) as smpool,
            tc.tile_pool(name="sden", bufs=2) as sdpool,
            tc.tile_pool(name="psA", bufs=3, space="PSUM") as psA,
            tc.tile_pool(name="psB", bufs=1, space="PSUM") as psB,
            tc.tile_pool(name="psC", bufs=2, space="PSUM") as psC,
            tc.tile_pool(name="dram", bufs=1, space="DRAM") as dram,
        ):
            # ---------- constants ----------
            wd0 = cpool.tile([D, N * D], BF16, tag="wd0")
            wd1 = cpool.tile([D, N * D], BF16, tag="wd1")
            nc.sync.dma_start(wd0[:].rearrange("d (n e) -> d n e", n=N),
                              xTview(wd0_e, "wd0"))
            nc.sync.dma_start(wd1[:].rearrange("d (n e) -> d n e", n=N),
                              xTview(wd1_e, "wd1"))
            ws0 = cpool.tile([D, D], BF16, tag="ws0")
            ws1 = cpool.tile([D, D], BF16, tag="ws1")
            nc.sync.dma_start(ws0[:], ws0_e[:])
            nc.sync.dma_start(ws1[:], ws1_e[:])
            b0 = cpool.tile([D, N], F32, tag="b0")
            b1 = cpool.tile([D, 1], F32, tag="b1")
            nc.sync.dma_start(b0[:], b0_e[:])
            nc.sync.dma_start(b1[:], b1_e[:])
            wg = cpool.tile([D, E], F32, tag="wg")
            oh = cpool.tile([B, E], F32, tag="oh")
            id32 = cpool.tile([B, B], F32, tag="id32")
            pmask = cpool.tile([D, 32], BF16, tag="pmask")
            nc.sync.dma_start(wg[:], wg_e[:])
            nc.sync.dma_start(oh[:], oh_e[:])
            nc.sync.dma_start(id32[:], id32_e[:])
            nc.sync.dma_start(pmask[:], pmask_e[:])

            g_bcast = cpool.tile([D, BT], BF16, tag="gb")
            xacc = cpool.tile([D, BT], BF16, tag="xacc")
            xacc2 = cpool.tile([D, BT], BF16, tag="xacc2")
            nc.vector.memset(xacc[:], 0.0)
            nc.vector.memset(xacc2[:], 0.0)

            # ---------- DRAM scratch ----------
            s_drams = [dram.tile([H, B, T], F32, name=f"sd{n}")
                       for n in range(N)]
            r_drams = [dram.tile([H, B, T], BF16, name=f"rd{n}")
                       for n in range(N)]
            g_dram = dram.tile([1, BT], BF16)
            e_drams = [dram.tile([8, D, BT], BF16, name=f"ed{j}")
                       for j in range(4)]
            slins = [dram.tile([D, 512], F32, name=f"sl{n}") for n in range(N)]
            rs_out = [dram.tile([D, BT], BF16, name=f"rs{j}") for j in range(4)]

            # ---------- gating: dedicated x re-read so gates are ready early
            for n in range(N):
                xa = xapool.tile([D, BT], BF16, tag="xa")
                nc.gpsimd.dma_start(xa[:], xT_e[n])
                acc = xacc if n % 2 == 0 else xacc2
                nc.vector.tensor_add(acc[:], acc[:], xa[:])
            nc.vector.tensor_add(xacc[:], xacc[:], xacc2[:])
            xsum = smpool.tile([D, B], F32, tag="xsum")
            nc.vector.reduce_sum(xsum[:],
                                 xacc[:].rearrange("d (b t) -> d b t", b=B),
                                 axis=mybir.AxisListType.X)
            psL = psB.tile([B, E], F32, tag="pssmall")
            nc.tensor.matmul(psL[:], xsum[:], wg[:], start=True, stop=True)
            ls = smpool.tile([B, E], F32, tag="ls")
            nc.vector.tensor_copy(ls[:], psL[:])
            m1 = smpool.tile([B, 1], F32, tag="m1")
            nc.vector.reduce_max(m1[:], ls[:], axis=mybir.AxisListType.X)
            mask1 = smpool.tile([B, E], F32, tag="mask1")
            nc.vector.tensor_scalar(mask1[:], ls[:], m1[:], None, op0=ALU.is_ge)
            lsm = smpool.tile([B, E], F32, tag="lsm")
            nc.vector.scalar_tensor_tensor(lsm[:], mask1[:], -1e30, ls[:],
                                           op0=ALU.mult, op1=ALU.add)
            m2 = smpool.tile([B, 1], F32, tag="m2")
            nc.vector.reduce_max(m2[:], lsm[:], axis=mybir.AxisListType.X)
            d21 = smpool.tile([B, 1], F32, tag="d21")
            nc.vector.tensor_sub(d21[:], m2[:], m1[:])
            ed = smpool.tile([B, 1], F32, tag="ed")
            nc.scalar.activation(ed[:], d21[:], AF.Exp)
            den = smpool.tile([B, 1], F32, tag="den")
            nc.vector.tensor_scalar_add(den[:], ed[:], 1.0)
            rden = smpool.tile([B, 1], F32, tag="rden")
            nc.vector.reciprocal(rden[:], den[:])
            m1n = smpool.tile([B, 1], F32, tag="m1n")
            nc.vector.tensor_scalar_mul(m1n[:], m1[:], -1.0)
            esh = smpool.tile([B, E], F32, tag="esh")
            nc.scalar.activation(esh[:], ls[:], AF.Exp, bias=m1n[:])
            g0 = smpool.tile([B, E], F32, tag="g0")
            nc.vector.tensor_scalar(g0[:], esh[:], rden[:], None, op0=ALU.mult)
            mask2 = smpool.tile([B, E], F32, tag="mask2")
            nc.vector.tensor_scalar(mask2[:], ls[:], m2[:], None, op0=ALU.is_ge)
            gg = smpool.tile([B, E], F32, tag="gg")
            nc.vector.tensor_mul(gg[:], g0[:], mask2[:])
            gsel0 = smpool.tile([B, E], F32, tag="gsel0")
            nc.vector.tensor_mul(gsel0[:], gg[:], oh[:])
            gsel = smpool.tile([B, 1], F32, tag="gsel")
            nc.vector.reduce_sum(gsel[:], gsel0[:], axis=mybir.AxisListType.X)
            psG = psB.tile([1, B], F32, tag="pssmall")
            nc.tensor.matmul(psG[:], gsel[:], id32[:], start=True, stop=True)
            grow = smpool.tile([1, B], BF16, tag="grow")
            nc.vector.tensor_copy(grow[:], psG[:])
            # expand each g[b] across its 64 t-columns (free-dim broadcast)
            growx = smpool.tile([1, BT], BF16, tag="growx")
            nc.vector.tensor_copy(
                growx[:].rearrange("p (b t) -> p b t", b=B),
                grow[:].unsqueeze(2).broadcast_to([1, B, T]))
            nc.gpsimd.dma_start(g_dram[:], growx[:])
            # broadcast over 128 partitions (contiguous inner run)
            nc.gpsimd.dma_start(
                g_bcast[:],
                g_dram[:].partition_broadcast(D).squeeze(1))

            # ---------- main per-variable pipeline ----------
            for n in range(N):
                xn = xnpool.tile([D, BT], BF16, tag="xn")
                nc.sync.dma_start(xn[:], xT_e[n])

                # k projection: kT = Wd0^T x -> [e_out, bt]
                kT = kvpool.tile([D, BT], BF16, tag="kT")
                for c in range(4):
                    psK = psA.tile([D, 512], F32, tag="ps512")
                    nc.tensor.matmul(psK[:], wd0[:, n * D:(n + 1) * D],
                                     xn[:, c * 512:(c + 1) * 512],
                                     start=True, stop=True)
                    nc.scalar.copy(kT[:, c * 512:(c + 1) * 512], psK[:])
                # v "projection", directly in [bt(2b), e] block layout:
                # stationary = xT chunk, moving = Wd1 slice
                vB = kvpool.tile([D, BT], BF16, tag="vB")
                for c in range(4):
                    psV = psA.tile([D, 512], F32, tag="ps512")
                    for u in range(4):
                        blk = c * 4 + u
                        nc.tensor.matmul(psV[:, u * 128:(u + 1) * 128],
                                         xn[:, blk * 128:(blk + 1) * 128],
                                         wd1[:, n * D:(n + 1) * D],
                                         start=True, stop=True)
                    nc.vector.tensor_copy(vB[:, c * 512:(c + 1) * 512], psV[:])

                # attention scores + softmax numerator, grouped 8 b's at a time
                pt = attpool.tile([D, BT], BF16, tag="pt")
                psS2 = psB.tile([D, 512], F32, tag="pssmall")
                for grp in range(4):
                    # h0 in bank 0 (cols 0:256), h1 in bank 1 (cols 512:768):
                    # different PE row-groups must write different PSUM banks
                    psS = psC.tile([D, 1024], F32, tag="ps1024")
                    for pig in range(4):
                        b0i = grp * 8 + pig * 2
                        for par in range(2):
                            b = b0i + par
                            for h in range(2):
                                # S^T[kk,q] = k_slice.T @ q_slice
                                nc.tensor.matmul(
                                    psS[par * 64:(par + 1) * 64,
                                        h * 512 + pig * 64:h * 512 + (pig + 1) * 64],
                                    kT[h * 64:(h + 1) * 64, b * 64:(b + 1) * 64],
                                    xn[h * 64:(h + 1) * 64, b * 64:(b + 1) * 64],
                                    start=True, stop=True)
                    # exp(S/sqrt(hd)) -> bf16; pt cols: grp*512 + h*256 + pig*64
                    nc.scalar.activation(
                        pt[:, grp * 512:(grp + 1) * 512]
                        .rearrange("d (hh c) -> d hh c", hh=2),
                        psS[:].rearrange("d (hh bk c) -> d (hh bk) c", hh=2, bk=2, c=256)
                        [:, 0::2, :],
                        AF.Exp, scale=0.125)
                    # softmax denominators: column sums per parity, packed at
                    # PSUM partition pairs {0,1},{32,33},{64,65},{96,97}
                    nc.tensor.matmul(psS2[32 * grp:32 * (grp + 1), :],
                                     pmask[:],
                                     pt[:, grp * 512:(grp + 1) * 512],
                                     start=True, stop=True,
                                     tile_position=(0, 32 * grp))

                # lane-locked copy out of PSUM, spill linearly, then
                # DRAM->DRAM gather (arbitrary strides allowed there)
                ssb = sdpool.tile([D, 512], F32, tag="ssb")
                nc.vector.tensor_copy(ssb[:], psS2[:])
                nc.gpsimd.dma_start(slins[n][:], ssb[:])
                sview = slins[n][:].rearrange("(g s) (hh i q) -> g s hh i q",
                                              s=32, hh=2, i=4)
                dview = s_drams[n][:].rearrange("hh (g i r) t -> hh g i r t",
                                                g=4, i=4)
                for h in range(2):
                    for r in range(2):
                        nc.gpsimd.dma_start(dview[h, :, :, r, :],
                                            sview[:, r, h, :, :])
                sg = sdpool.tile([D, B], F32, tag="sg")
                nc.gpsimd.dma_start(
                    sg[:], s_drams[n][:].rearrange("h b t -> (h b t)")
                    .rearrange("(p c) -> p c", c=B))
                rsb = sdpool.tile([D, B], BF16, tag="rsb")
                nc.vector.reciprocal(rsb[:], sg[:])
                nc.gpsimd.dma_start(
                    r_drams[n][:].rearrange("h b t -> (h b t)")
                    .rearrange("(p c) -> p c", c=B), rsb[:])
                rbc = eppool.tile([D, BT], BF16, tag="rbc")
                for h in range(2):
                    nc.sync.dma_start(
                        rbc[h * 64:(h + 1) * 64, :],
                        r_drams[n][h].rearrange("b t -> (b t)")
                        .unsqueeze(0).partition_broadcast(64).squeeze(1))

                # att @ v -> unnormalized o^T, then normalize via rbc
                oT = mlppool.tile([D, BT], BF16, tag="oT")
                for grp in range(4):
                    # par0 in bank 0, par1 in bank 1 (row-group = parity here)
                    psO = psC.tile([D, 1024], F32, tag="ps1024")
                    for pig in range(4):
                        for par in range(2):
                            for h in range(2):
                                nc.tensor.matmul(
                                    psO[h * 64:(h + 1) * 64,
                                        par * 512 + pig * 64:par * 512 + (pig + 1) * 64],
                                    vB[par * 64:(par + 1) * 64,
                                       (grp * 4 + pig) * 128 + h * 64:
                                       (grp * 4 + pig) * 128 + (h + 1) * 64],
                                    pt[par * 64:(par + 1) * 64,
                                       grp * 512 + h * 256 + pig * 64:
                                       grp * 512 + h * 256 + (pig + 1) * 64],
                                    start=True, stop=True)
                    for par in range(2):
                        nc.vector.scalar_tensor_tensor(
                            oT[:, grp * 512:(grp + 1) * 512]
                            .rearrange("d (pp rr q) -> d rr pp q",
                                       pp=4, rr=2)[:, par],
                            psO[:, par * 512:par * 512 + 256]
                            .rearrange("d (pp q) -> d pp q", pp=4),
                            0.0,
                            rbc[:, grp * 512:(grp + 1) * 512]
                            .rearrange("d (pp rr q) -> d rr pp q",
                                       pp=4, rr=2)[:, par],
                            op0=ALU.add, op1=ALU.mult)

                # MLP in f32r: U1 = ws0.T@oT, relu(+b0[n]); U2 = ws1.T@o1
                o1 = mlppool.tile([D, BT], BF16, tag="o1")
                for c in range(4):
                    psU = psA.tile([D, 512], F32, tag="ps512")
                    nc.tensor.matmul(psU[:], ws0[:],
                                     oT[:, c * 512:(c + 1) * 512],
                                     start=True, stop=True)
                    if c % 2 == 0:
                        nc.scalar.activation(o1[:, c * 512:(c + 1) * 512],
                                             psU[:], AF.Relu,
                                             bias=b0[:, n:n + 1])
                    else:
                        nc.vector.tensor_scalar(o1[:, c * 512:(c + 1) * 512],
                                                psU[:], b0[:, n:n + 1], 0.0,
                                                op0=ALU.add, op1=ALU.max)
                dt_ = dpool.tile([D, BT], BF16, tag="dt")
                for c in range(4):
                    psU2 = psA.tile([D, 512], F32, tag="ps512")
                    nc.tensor.matmul(psU2[:], ws1[:],
                                     o1[:, c * 512:(c + 1) * 512],
                                     start=True, stop=True)
                    # t = exp(U2 + b1); d = t - 1  (in bf16, values ~ +-0.05)
                    tx = eppool.tile([D, 512], F32, tag="tx")
                    nc.scalar.activation(tx[:], psU2[:], AF.Exp, bias=b1[:])
                    nc.vector.tensor_scalar_sub(dt_[:, c * 512:(c + 1) * 512],
                                                tx[:], 1.0)
                # E = d * g  (gating-dependent; Tile stalls this op until ready)
                ep = eppool.tile([D, BT], BF16, tag="ep")
                nc.vector.tensor_mul(ep[:], dt_[:], g_bcast[:])
                nc.sync.dma_start(e_drams[n // 8][n % 8], ep[:])

                # chunked ReduceScatter after each 8-n block
                if n % 8 == 7:
                    j = n // 8
                    nc.gpsimd.collective_compute(
                        "ReduceScatter", ALU.add,
                        replica_groups=[CORE_IDS],
                        ins=[e_drams[j][:].opt()],
                        outs=[rs_out[j].opt()],
                    )

            # consume RS results at the very end so nothing queues behind
            # the collective waits in any engine stream
            for j in range(4):
                cmb = eppool.tile([D, BT], BF16, tag="cmb")
                nc.sync.dma_start(cmb[:], rs_out[j][:])
                lg = eppool.tile([D, BT], F32, tag="lg")
                nc.scalar.activation(lg[:], cmb[:], AF.Ln, bias=1.0)
                nc.sync.dma_start(out_e[j], lg[:])

    nc.finalize()
    return nc


def xTview(t, _name):
    return t[:].rearrange("n d e -> d n e")


def prep_inputs(x, Wg, Wd, bd, Ws, bs):
    """Host-side sharding/layout prep. Returns in_maps for the 8 cores."""
    xT = np.ascontiguousarray(
        x.astype(np.float32).transpose(2, 3, 0, 1).reshape(N, D, BT)
    ).astype(ml_dtypes.bfloat16)
    wg_s = (Wg.astype(np.float32) / np.float32(T * N)).astype(np.float32)
    id32 = np.eye(B, dtype=np.float32)
    pmask = np.zeros((D, 32), dtype=ml_dtypes.bfloat16)
    pmask[:64, 0::2] = 1
    pmask[64:, 1::2] = 1
    in_maps = []
    for e in range(E):
        wd0 = np.ascontiguousarray(Wd[e, 0]).astype(ml_dtypes.bfloat16)
        wd1 = np.ascontiguousarray(Wd[e, 1]).astype(ml_dtypes.bfloat16)
        ws0 = np.ascontiguousarray(Ws[e, 0]).astype(ml_dtypes.bfloat16)
        ws1 = np.ascontiguousarray(Ws[e, 1]).astype(ml_dtypes.bfloat16)
        # fold value-bias through Ws0 (softmax rows sum to 1)
        b0 = (bs[e, 0] + bd[e, 1] @ Ws[e, 0]).astype(np.float32).T  # [D, N]
        b1 = bs[e, 1].astype(np.float32).reshape(D, 1)
        oh = np.zeros((B, E), dtype=np.float32)
        oh[:, e] = 1.0
        in_maps.append({
            "xT": xT, "wd0": wd0, "wd1": wd1,
            "ws0": ws0, "ws1": ws1, "b0": np.ascontiguousarray(b0), "b1": b1,
            "wg": wg_s, "oh": oh, "id32": id32, "pmask": pmask,
        })
    return in_maps


def kernel(x, Wg, Wd, bd, Ws, bs, _trace=False):
    if "nc" not in _cache:
        _cache["nc"] = build()
    nc = _cache["nc"]
    in_maps = prep_inputs(np.asarray(x), np.asarray(Wg), np.asarray(Wd),
                          np.asarray(bd), np.asarray(Ws), np.asarray(bs))
    res = run_bass_kernel_spmd(nc, in_maps, CORE_IDS, trace=_trace)
    # reassemble: core i's chunk j is variable n = j*8 + i, layout [D, B*T]
    out_T = np.empty((N, D, B, T), dtype=np.float32)
    for i in range(N_CORES):
        o = res.results[i]["out"].reshape(N // N_CORES, D, B, T)
        for j in range(N // N_CORES):
            out_T[j * 8 + i] = o[j]
    out = out_T.transpose(2, 3, 0, 1)  # [B, T, N, D]
    if _trace:
        kernel.last_exec_ns = res.exec_time_ns
    return np.ascontiguousarray(out)
